# revision 8
# baseline (speedup 1.0000x reference)
"""DeepseekV2 MoE block on 8 Trainium2 NeuronCores.

Strategy: data-parallel over tokens (2048 tokens/core), all expert weights
replicated per core (fp16), fully on-device routing:
  router (2-pass stacked fp16 hi/lo matmul, all 4 cross terms kept)
  -> top-2 via DVE max/max_index, weights via exp/reciprocal
  -> per-expert slot positions via strict-triangular-matmul prefix sums
  -> (token_id+1, weight) records scatter-added into a DRAM slot table
  -> per-expert dma_gather(transpose=True) dispatch (H on partitions)
  -> per-expert gate/up/gelu/mul/down matmuls over CC=576 of C=640 slots,
     gating applied as per-partition scalar on the slot-major down output
  -> expert outputs written slot-major to an eout DRAM table (plain DMA,
     no scatter-add, no serialization)
  -> dense shared expert
  -> combine: per-token dma_gather of the 2 pre-weighted expert rows from
     eout + shared add -> y (fp16, widened to fp32 on host).
"""

import numpy as np
from contextlib import ExitStack

import concourse.bass as bass
import concourse.tile as tile
from concourse import bacc, mybir, library_config
from concourse.bass_utils import run_bass_kernel_spmd

F16 = mybir.dt.float16
F32 = mybir.dt.float32
I16 = mybir.dt.int16
I32 = mybir.dt.int32
U32 = mybir.dt.uint32

NCORES = 8
B, S, H, I, E, K = 4, 4096, 1024, 512, 8, 2
T = B * S                 # 16384 global tokens
TL = T // NCORES          # 2048 tokens per core
NT = TL // 128            # 16 token tiles
HC = H // 128             # 8 H chunks
IC = I // 128             # 4 I chunks
C = 640                   # per-expert slot stride (gather num_idxs: %128==0)
CC = 576                  # computed slot width (max observed load 568)
SC = C // 128             # 5 slot chunks per expert (slot-table stride)
NSLOT = E * C             # 5120
REC_F = 64                # record row = 64 fp32 = 256 B (min: 256B granule)
AF = mybir.ActivationFunctionType
ALU = mybir.AluOpType

# down-proj slot chunks covering CC: (start, n)
CCHUNKS = [(0, 128), (128, 128), (256, 128), (384, 128), (512, 64)]
# gate/up free-dim splits covering CC (PSUM bank is 512 f32)
GUSPLIT = [(0, 512), (512, 64)]


def _build_program(loop_n=1):
    nc = bacc.Bacc("TRN2", target_bir_lowering=False, debug=False)

    d = {}
    def din(name, shape, dtype):
        d[name] = nc.dram_tensor(name, list(shape), dtype, kind="ExternalInput")
        return d[name]

    # per-core activations
    din("xt_hi", (HC, 128, TL), F16)      # xT hi chunks: [hc, p, t] = x[t, hc*128+p]
    din("xt_lo", (HC, 128, TL), F16)
    din("x_hi", (TL, H), F16)             # token-major gather table
    # router weights
    din("gwt_hi", (HC, 128, E), F16)
    din("gwt_lo", (HC, 128, E), F16)
    # expert weights (lhsT layouts)
    din("wg", (E, 128, HC, IC, 128), F16)  # [e,p,hc,ic,m] = Wg[e, hc*128+p, ic*128+m]
    din("wu", (E, 128, HC, IC, 128), F16)
    din("wd", (E, 128, IC, H), F16)        # [e,p,ic,:] = Wd[e, ic*128+p, :]
    din("swg", (128, HC, IC, 128), F16)
    din("swu", (128, HC, IC, 128), F16)
    din("swd", (128, IC, H), F16)
    # constants
    din("ident", (128, 128), F32)
    din("tri", (128, 128), F32)            # tri[k, m] = 1.0 if k < m else 0
    din("repsel", (8, 128, 128), F32)      # repsel[r, p, m] = (p == (m%16)+16r)
    din("iota1", (128, NT), F32)           # [p, i] = i*128 + p + 1

    y_d = nc.dram_tensor("y", [TL, H], F16, kind="ExternalOutput")
    srec = nc.dram_tensor("srec", [NSLOT, REC_F], F32)    # internal
    eout = nc.dram_tensor("eout", [NSLOT, H], F16)        # internal slot table

    with tile.TileContext(nc) as tc:
        if loop_n > 1:
            with tc.For_i(0, loop_n, 1):
                _moe(tc, d, y_d, srec, eout)
        else:
            _moe(tc, d, y_d, srec, eout)
    nc.compile()
    return nc


def _moe(tc, d, y_d, srec, eout):
    nc = tc.nc

    with ExitStack() as ctx:
        nc.gpsimd.load_library(library_config.mlp)

        const = ctx.enter_context(tc.tile_pool(name="const", bufs=1))
        p_keep = ctx.enter_context(tc.tile_pool(name="keep", bufs=1))
        p_ysh = ctx.enter_context(tc.tile_pool(name="ysh", bufs=1))
        # PSUM budget is 8 banks of 2 KB/partition total:
        #   p_gu: gate+up accumulators, 2 tags x [128,768] f32 = 4 banks
        #   p_dn: universal pool, 2 bufs x [128,1024] f32 = 4 banks
        p_gu = ctx.enter_context(tc.tile_pool(name="psgu", bufs=1, space="PSUM"))
        p_dn = ctx.enter_context(tc.tile_pool(name="psdn", bufs=2, space="PSUM"))

        _ctr = [0]

        def ps_uni():
            _ctr[0] += 1
            return p_dn.tile([128, 1024], F32, tag="uni", name=f"uni{_ctr[0]}")

        # stacked router weights: cols 0:8 = hi, 8:16 = lo
        gwt = const.tile([128, HC, 2 * E], F16)
        nc.sync.dma_start(gwt[:, :, :E], d["gwt_hi"].ap().rearrange("hc p e -> p hc e"))
        nc.sync.dma_start(gwt[:, :, E:], d["gwt_lo"].ap().rearrange("hc p e -> p hc e"))

        # shared-expert tensors that must survive into phase 10
        p_swd = ctx.enter_context(tc.tile_pool(name="swd", bufs=1))
        p_sint = ctx.enter_context(tc.tile_pool(name="sint", bufs=1))

        # routing outputs that must survive into the expert/combine phases
        wrapA = p_keep.tile([128, 128], I16)
        wrapB = p_keep.tile([128, 128], I16)
        wrapD = p_keep.tile([128, NSLOT // 16], I16)
        w_slot = p_keep.tile([128, NSLOT // 128], F32)
        ysh = p_ysh.tile([128, NT, H], F16)

        def fold_wrap16(pool, src, ncols, dst_i16):
            """src [128, ncols] f32 with element j at [j%128, j//128] ->
            dst_i16 [128, 8*ncols] int16 wrap16: element j at [j%16, j//16],
            replicated across partition groups of 16."""
            w3 = pool.tile([128, ncols, 8], F32, tag=f"w3_{ncols}",
                           name=f"w3_{ncols}_{_ctr[0]}")
            for r in range(8):
                ps_f = ps_uni()[:, :ncols]
                nc.tensor.matmul(ps_f[:], repsel[:, r, :], src[:],
                                 start=True, stop=True)
                nc.vector.tensor_copy(w3[:, :, r], ps_f[:])
            nc.vector.tensor_copy(dst_i16[:],
                                  w3[:].rearrange("p a b -> p (a b)"))

        with ExitStack() as xctx:
            p_xt = xctx.enter_context(tc.tile_pool(name="xt", bufs=1))
            p_sw = xctx.enter_context(tc.tile_pool(name="swp", bufs=1))
            xt_hi = p_xt.tile([128, HC, TL], F16)
            for hc in range(HC):
                nc.sync.dma_start(xt_hi[:, hc, :], d["xt_hi"].ap()[hc])

            ident = const.tile([128, 128], F32)
            nc.sync.dma_start(ident[:], d["ident"].ap())
            tri = const.tile([128, 128], F32)
            nc.sync.dma_start(tri[:], d["tri"].ap())
            repsel = const.tile([128, 8, 128], F32)
            nc.sync.dma_start(repsel[:], d["repsel"].ap().rearrange("r p m -> p r m"))
            iota1 = const.tile([128, NT], F32)
            nc.sync.dma_start(iota1[:], d["iota1"].ap())

            with ExitStack() as rctx:
                p_xtlo = rctx.enter_context(tc.tile_pool(name="xtlo", bufs=1))
                p_rt = rctx.enter_context(tc.tile_pool(name="rt", bufs=1))
                xt_lo = p_xtlo.tile([128, HC, TL], F16)
                for hc in range(HC):
                    nc.sync.dma_start(xt_lo[:, hc, :], d["xt_lo"].ap()[hc])

                swg_sb = p_sw.tile([128, HC, IC, 128], F16)
                nc.sync.dma_start(swg_sb[:], d["swg"].ap())
                swu_sb = p_sw.tile([128, HC, IC, 128], F16)
                nc.sync.dma_start(swu_sb[:], d["swu"].ap())
                swd_sb = p_swd.tile([128, IC, H], F16)
                nc.sync.dma_start(swd_sb[:], d["swd"].ap())

                # ---- Phase 1: router logits [2E, TL] ----
                # 2 stacked passes: psum rows 0:8 get (ghi|glo)@xhi, rows 8:16
                # get (ghi|glo)@xlo; all 4 hi/lo terms are kept — the halves
                # are summed after the phase-2 transpose (free-dim add).
                logit_sb = p_rt.tile([2 * E, TL], F32)
                for ntile in range(TL // 512):
                    ps_log = ps_uni()[:2 * E, :512]
                    sl = slice(ntile * 512, (ntile + 1) * 512)
                    for pi, x_t in enumerate((xt_hi, xt_lo)):
                        for hc in range(HC):
                            nc.tensor.matmul(
                                ps_log[:], gwt[:, hc, :], x_t[:, hc, sl],
                                start=(hc == 0 and pi == 0),
                                stop=(hc == HC - 1 and pi == 1))
                    nc.vector.tensor_copy(logit_sb[:, sl], ps_log[:])

                # ---- Phase 2: transpose -> token-major, sum hi/lo halves ----
                L2 = p_rt.tile([128, NT, 2, E], F32)
                for i in range(NT):
                    ps_t = ps_uni()[:, :2 * E]
                    nc.tensor.transpose(ps_t[:],
                                        logit_sb[:, i * 128:(i + 1) * 128],
                                        ident[:2 * E, :2 * E])
                    nc.vector.tensor_copy(L2[:, i, :, :], ps_t[:])
                L = p_rt.tile([128, NT, 8], F32)
                nc.vector.tensor_tensor(L[:], L2[:, :, 0, :], L2[:, :, 1, :],
                                        ALU.add)

                # ---- Phase 3: top-2 + gate weights ----
                v8 = p_rt.tile([128, NT, 8], F32)
                i8 = p_rt.tile([128, NT, 8], U32)
                for i in range(NT):
                    nc.vector.max(v8[:, i], L[:, i])
                    nc.vector.max_index(i8[:, i], v8[:, i], L[:, i])
                w1 = p_rt.tile([128, NT], F32)
                w2 = p_rt.tile([128, NT], F32)
                zt = p_rt.tile([128, NT], F32)
                # z = exp(v2 - v1); w1 = 1/(1+z); w2 = 1 - w1
                nc.vector.tensor_tensor(zt[:], v8[:, :, 1], v8[:, :, 0],
                                        ALU.subtract)
                nc.scalar.activation(zt[:], zt[:], AF.Exp)
                nc.vector.tensor_scalar_add(zt[:], zt[:], 1.0)
                nc.vector.reciprocal(w1[:], zt[:])
                nc.vector.tensor_scalar(w2[:], w1[:], -1.0, 1.0, ALU.mult,
                                        ALU.add)
                e1f = p_rt.tile([128, NT], F32)
                e2f = p_rt.tile([128, NT], F32)
                nc.vector.tensor_copy(e1f[:], i8[:, :, 0])
                nc.vector.tensor_copy(e2f[:], i8[:, :, 1])

                # ---- Phase 4: masks + prefix-sum positions ----
                C1 = p_rt.tile([128, E, NT], F32)
                C2 = p_rt.tile([128, E, NT], F32)
                M = p_rt.tile([128, E, NT], F32)
                for e in range(E):
                    nc.vector.tensor_scalar(C1[:, e], e1f[:], float(e), None,
                                            ALU.is_equal)
                    nc.vector.tensor_scalar(C2[:, e], e2f[:], float(e), None,
                                            ALU.is_equal)
                    nc.vector.tensor_tensor(M[:, e], C1[:, e], C2[:, e], ALU.add)
                rowsum = p_rt.tile([128, E], F32)
                nc.vector.tensor_reduce(rowsum[:], M[:], mybir.AxisListType.X,
                                        ALU.add)

                # carry[p, e] = sum_{k<p} rowsum[k, e]
                ps_carry = ps_uni()[:, :8]
                nc.tensor.matmul(ps_carry[:], tri[:], rowsum[:], start=True,
                                 stop=True)
                carry = p_rt.tile([128, E], F32)
                nc.vector.tensor_copy(carry[:], ps_carry[:])

                # exclusive scan over i (Hillis-Steele, ping-pong)
                S0 = p_rt.tile([128, E, NT], F32)
                S1 = p_rt.tile([128, E, NT], F32)
                nc.vector.tensor_copy(S0[:], M[:])
                a, b = S0, S1
                for s in (1, 2, 4, 8):
                    nc.vector.tensor_copy(b[:, :, :s], a[:, :, :s])
                    nc.vector.tensor_tensor(b[:, :, s:], a[:, :, s:],
                                            a[:, :, :NT - s], ALU.add)
                    a, b = b, a
                pos = p_rt.tile([128, E, NT], F32)
                nc.vector.tensor_tensor(pos[:], a[:], M[:], ALU.subtract)
                nc.vector.tensor_tensor(
                    pos[:], pos[:],
                    carry[:, :, None].to_broadcast([128, E, NT]), ALU.add)

                pos1 = p_rt.tile([128, NT], F32)
                pos2 = p_rt.tile([128, NT], F32)
                tmp = p_rt.tile([128, NT], F32)
                nc.vector.memset(pos1[:], 0.0)
                nc.vector.memset(pos2[:], 0.0)
                for e in range(E):
                    nc.vector.tensor_tensor(tmp[:], pos[:, e], C1[:, e], ALU.mult)
                    nc.vector.tensor_tensor(pos1[:], pos1[:], tmp[:], ALU.add)
                    nc.vector.tensor_tensor(tmp[:], pos[:, e], C2[:, e], ALU.mult)
                    nc.vector.tensor_tensor(pos2[:], pos2[:], tmp[:], ALU.add)
                idx1f = p_rt.tile([128, NT], F32)
                idx2f = p_rt.tile([128, NT], F32)
                nc.vector.tensor_scalar(idx1f[:], e1f[:], float(C), None,
                                        ALU.mult)
                nc.vector.tensor_tensor(idx1f[:], idx1f[:], pos1[:], ALU.add)
                nc.vector.tensor_scalar(idx2f[:], e2f[:], float(C), None,
                                        ALU.mult)
                nc.vector.tensor_tensor(idx2f[:], idx2f[:], pos2[:], ALU.add)

                # ---- Phase 5: wrap16 index lists for scatter/combine ----
                fold_wrap16(p_rt, idx1f, NT, wrapA)
                fold_wrap16(p_rt, idx2f, NT, wrapB)

                # ---- Phase 6: record scatter into srec ----
                zero_t = p_rt.tile([128, NSLOT * REC_F // 128], F32)
                nc.vector.memset(zero_t[:], 0.0)
                i_zero = nc.gpsimd.dma_start(
                    srec.ap().rearrange("(a b) f -> a (b f)", a=128), zero_t[:])

                recA = p_rt.tile([128, NT, REC_F], F32)
                recB = p_rt.tile([128, NT, REC_F], F32)
                nc.vector.memset(recA[:], 0.0)
                nc.vector.memset(recB[:], 0.0)
                nc.vector.tensor_scalar_add(recA[:, :, 0], iota1[:], 0.0)
                nc.vector.tensor_copy(recA[:, :, 1], w1[:])
                nc.vector.tensor_scalar_add(recB[:, :, 0], iota1[:], 0.0)
                nc.vector.tensor_copy(recB[:, :, 1], w2[:])

                i_scA = nc.gpsimd.dma_scatter_add(
                    srec.ap(), recA[:], wrapA[:], TL, TL, REC_F)
                i_scB = nc.gpsimd.dma_scatter_add(
                    srec.ap(), recB[:], wrapB[:], TL, TL, REC_F)
                tile.add_dep_helper(i_scA.ins, i_zero.ins,
                                    reason="zero before scatter")
                tile.add_dep_helper(i_scB.ins, i_zero.ins,
                                    reason="zero before scatter")

                # ---- Phase 7: readback, dispatch lists ----
                RB = p_rt.tile([128, NSLOT // 128, REC_F], F32)
                i_rb = nc.sync.dma_start(
                    RB[:], srec.ap().rearrange("(c p) f -> p c f", p=128))
                tile.add_dep_helper(i_rb.ins, i_scA.ins,
                                    reason="scatter before readback")
                tile.add_dep_helper(i_rb.ins, i_scB.ins,
                                    reason="scatter before readback")

                # empty slots hold 0 -> would become -1: clamp to token 0 so
                # the static-count dispatch gather never reads out of bounds
                # (row 0 garbage is computed, zero-gated, and never combined)
                t_slot = p_rt.tile([128, NSLOT // 128], F32)
                nc.vector.tensor_scalar(t_slot[:], RB[:, :, 0], -1.0, 0.0,
                                        ALU.add, ALU.max)
                nc.vector.tensor_copy(w_slot[:], RB[:, :, 1])
                fold_wrap16(p_rt, t_slot, NSLOT // 128, wrapD)

            # ---- Phase 8: shared expert gate/up (down-proj is phase 10) ----
            with ExitStack() as sctx:
                inter_s = p_sint.tile([128, IC, TL], F16)
                for ic in range(IC):
                    for q in range(4):
                        qs = slice(q * 512, (q + 1) * 512)
                        psg_f = p_gu.tile([128, 2, 512], F32, tag="g",
                                          name=f"psgs{ic}_{q}")
                        psu_f = p_gu.tile([128, 2, 512], F32, tag="u",
                                          name=f"psus{ic}_{q}")
                        ps_g = psg_f[:, 0, :]
                        ps_u = psu_f[:, 0, :]
                        for hc in range(HC):
                            nc.tensor.matmul(ps_g[:], swg_sb[:, hc, ic, :],
                                             xt_hi[:, hc, qs], start=(hc == 0),
                                             stop=(hc == HC - 1))
                        for hc in range(HC):
                            nc.tensor.matmul(ps_u[:], swu_sb[:, hc, ic, :],
                                             xt_hi[:, hc, qs], start=(hc == 0),
                                             stop=(hc == HC - 1))
                        gel = p_sint.tile([128, 512], F16, tag="sgel",
                                          name=f"sgel{ic}_{q}")
                        nc.scalar.activation(gel[:], ps_g[:], AF.Gelu)
                        nc.vector.tensor_tensor(inter_s[:, ic, qs], gel[:],
                                                ps_u[:], ALU.mult)

        # ---- Phase 9: routed experts ----
        eout_write_insts = []
        with ExitStack() as ectx:
            p_w = ectx.enter_context(tc.tile_pool(name="wexp", bufs=2))
            p_xe = ectx.enter_context(tc.tile_pool(name="xe", bufs=3))
            p_int = ectx.enter_context(tc.tile_pool(name="inter", bufs=2))
            p_out = ectx.enter_context(tc.tile_pool(name="eout", bufs=3))

            for e in range(E):
                wg_sb = p_w.tile([128, HC, IC, 128], F16, tag="wg",
                                 name=f"wg{e}")
                nc.sync.dma_start(wg_sb[:], d["wg"].ap()[e])
                wu_sb = p_w.tile([128, HC, IC, 128], F16, tag="wu",
                                 name=f"wu{e}")
                nc.sync.dma_start(wu_sb[:], d["wu"].ap()[e])
                wd_sb = p_w.tile([128, IC, H], F16, tag="wd", name=f"wd{e}")
                nc.sync.dma_start(wd_sb[:], d["wd"].ap()[e])

                xe = p_xe.tile([128, HC, C], F16, tag="xe", name=f"xe{e}")
                nc.gpsimd.dma_gather(
                    xe[:], d["x_hi"].ap(),
                    wrapD[:, e * (C // 16):(e + 1) * (C // 16)],
                    C, C, H, transpose=True)

                inter = p_int.tile([128, IC, CC], F16, tag="inter",
                                   name=f"inter{e}")
                for ic in range(IC):
                    ps_g = p_gu.tile([128, 2, 512], F32, tag="g",
                                     name=f"psg{e}_{ic}")
                    ps_u = p_gu.tile([128, 2, 512], F32, tag="u",
                                     name=f"psu{e}_{ic}")
                    for half, (h0, hn) in enumerate(GUSPLIT):
                        hs = slice(h0, h0 + hn)
                        for hc in range(HC):
                            nc.tensor.matmul(ps_g[:, half, :hn],
                                             wg_sb[:, hc, ic, :],
                                             xe[:, hc, hs], start=(hc == 0),
                                             stop=(hc == HC - 1))
                        for hc in range(HC):
                            nc.tensor.matmul(ps_u[:, half, :hn],
                                             wu_sb[:, hc, ic, :],
                                             xe[:, hc, hs], start=(hc == 0),
                                             stop=(hc == HC - 1))
                    gel = p_int.tile([128, CC], F16, tag="gel", name=f"gel{e}_{ic}")
                    for half, (h0, hn) in enumerate(GUSPLIT):
                        nc.scalar.activation(gel[:, h0:h0 + hn],
                                             ps_g[:, half, :hn], AF.Gelu)
                        nc.vector.tensor_tensor(
                            inter[:, ic, h0:h0 + hn], gel[:, h0:h0 + hn],
                            ps_u[:, half, :hn], ALU.mult)

                for ci, (s0, sn) in enumerate(CCHUNKS):
                    ps_d = ps_uni()
                    for ic in range(IC):
                        for half in range(2):
                            hs = slice(half * 512, (half + 1) * 512)
                            nc.tensor.matmul(
                                ps_d[:sn, hs],
                                inter[:, ic, s0:s0 + sn],
                                wd_sb[:, ic, hs], start=(ic == 0),
                                stop=(ic == IC - 1))
                    eo = p_out.tile([128, H], F16, tag="eo", name=f"eo{e}_{ci}")
                    nc.vector.tensor_scalar_mul(
                        eo[:sn, :], ps_d[:sn, :],
                        w_slot[:sn, e * SC + ci:e * SC + ci + 1])
                    i_w = nc.sync.dma_start(
                        eout.ap()[e * C + s0:e * C + s0 + sn, :], eo[:sn, :])
                    eout_write_insts.append(i_w)

        # ---- Phase 10: shared down-proj (overlaps combine gathers) + combine
        for i in range(NT):
            ps_d = ps_uni()
            for ic in range(IC):
                for half in range(2):
                    hs = slice(half * 512, (half + 1) * 512)
                    nc.tensor.matmul(
                        ps_d[:, hs],
                        inter_s[:, ic, i * 128:(i + 1) * 128],
                        swd_sb[:, ic, hs], start=(ic == 0),
                        stop=(ic == IC - 1))
            nc.vector.tensor_copy(ysh[:, i, :], ps_d[:])

        NCH = 4                       # combine chunks
        TCH = TL // NCH               # tokens per chunk
        with ExitStack() as cctx:
            p_cmb = cctx.enter_context(tc.tile_pool(name="cmb", bufs=2))
            y_view = y_d.ap().rearrange("(i p) h -> p i h", p=128)
            for ci in range(NCH):
                cs = slice(ci * (TCH // 128), (ci + 1) * (TCH // 128))
                wsl = slice(ci * (TCH // 16), (ci + 1) * (TCH // 16))
                gA = p_cmb.tile([128, TCH // 128, H], F16, tag="gA",
                                name=f"gA{ci}")
                gB = p_cmb.tile([128, TCH // 128, H], F16, tag="gB",
                                name=f"gB{ci}")
                i_gA = nc.gpsimd.dma_gather(
                    gA[:], eout.ap(), wrapA[:, wsl], TCH, TCH, H)
                i_gB = nc.gpsimd.dma_gather(
                    gB[:], eout.ap(), wrapB[:, wsl], TCH, TCH, H)
                for i_w in eout_write_insts:
                    tile.add_dep_helper(i_gA.ins, i_w.ins,
                                        reason="eout writes before gather")
                    tile.add_dep_helper(i_gB.ins, i_w.ins,
                                        reason="eout writes before gather")
                nc.vector.tensor_tensor(gA[:], gA[:], gB[:], ALU.add)
                nc.vector.tensor_tensor(gA[:], gA[:], ysh[:, cs, :], ALU.add)
                nc.sync.dma_start(y_view[:, cs, :], gA[:])


_PROG = None


def _get_program():
    global _PROG
    if _PROG is None:
        _PROG = _build_program()
    return _PROG


def _split_hi_lo(x):
    hi = x.astype(np.float16)
    lo = (x - hi.astype(np.float32)).astype(np.float16)
    return hi, lo


def _make_consts():
    ident = np.eye(128, dtype=np.float32)
    k = np.arange(128)
    tri = (k[:, None] < k[None, :]).astype(np.float32)
    m = np.arange(128)
    repsel = np.zeros((8, 128, 128), np.float32)
    for r in range(8):
        repsel[r] = (k[:, None] == (m[None, :] % 16) + 16 * r)
    iota1 = (np.arange(NT)[None, :] * 128 + k[:, None] + 1).astype(np.float32)
    return ident, tri, repsel, iota1


def prepare_in_maps(hidden_states, gate_w, Wg, Wu, Wd, sWg, sWu, sWd):
    x = np.ascontiguousarray(np.asarray(hidden_states, np.float32).reshape(T, H))
    gw = np.asarray(gate_w, np.float32)
    gw_hi, gw_lo = _split_hi_lo(gw)

    wg_l = np.ascontiguousarray(
        np.asarray(Wg, np.float32).astype(np.float16)
        .reshape(E, HC, 128, IC, 128).transpose(0, 2, 1, 3, 4))
    wu_l = np.ascontiguousarray(
        np.asarray(Wu, np.float32).astype(np.float16)
        .reshape(E, HC, 128, IC, 128).transpose(0, 2, 1, 3, 4))
    wd_l = np.ascontiguousarray(
        np.asarray(Wd, np.float32).astype(np.float16)
        .reshape(E, IC, 128, H).transpose(0, 2, 1, 3))
    swg_l = np.ascontiguousarray(
        np.asarray(sWg, np.float32).astype(np.float16)
        .reshape(HC, 128, IC, 128).transpose(1, 0, 2, 3))
    swu_l = np.ascontiguousarray(
        np.asarray(sWu, np.float32).astype(np.float16)
        .reshape(HC, 128, IC, 128).transpose(1, 0, 2, 3))
    swd_l = np.ascontiguousarray(
        np.asarray(sWd, np.float32).astype(np.float16)
        .reshape(IC, 128, H).transpose(1, 0, 2))

    gwt_hi = np.ascontiguousarray(gw_hi.T.reshape(HC, 128, E))
    gwt_lo = np.ascontiguousarray(gw_lo.T.reshape(HC, 128, E))
    ident, tri, repsel, iota1 = _make_consts()

    shared = dict(gwt_hi=gwt_hi, gwt_lo=gwt_lo, wg=wg_l, wu=wu_l, wd=wd_l,
                  swg=swg_l, swu=swu_l, swd=swd_l, ident=ident, tri=tri,
                  repsel=repsel, iota1=iota1)

    in_maps = []
    for c in range(NCORES):
        xs = x[c * TL:(c + 1) * TL]
        hi, lo = _split_hi_lo(xs)
        xt_hi = np.ascontiguousarray(hi.T.reshape(HC, 128, TL))
        xt_lo = np.ascontiguousarray(lo.T.reshape(HC, 128, TL))
        in_maps.append(dict(shared, xt_hi=xt_hi, xt_lo=xt_lo,
                            x_hi=np.ascontiguousarray(hi)))
    return in_maps


def kernel(hidden_states, gate_w, Wg, Wu, Wd, sWg, sWu, sWd):
    nc = _get_program()
    in_maps = prepare_in_maps(hidden_states, gate_w, Wg, Wu, Wd, sWg, sWu, sWd)
    res = run_bass_kernel_spmd(nc, in_maps, list(range(NCORES)))
    y = np.concatenate([res.results[c]["y"] for c in range(NCORES)], axis=0)
    return y.reshape(B, S, H).astype(np.float32)
